# revision 31
# baseline (speedup 1.0000x reference)
"""AnomalyScorer Trainium2 kernel (8 NeuronCores, SPMD edge-parallel).

Strategy:
  - Host folds the per-feature scales a/b into two tables (ha = h*a, hb = h*b),
    so the device only needs gather + add + square-reduce + sigmoid + weight.
  - Edges are sharded across 8 cores (37500 each, padded to 37504 = 128*293).
  - Per core, only ~31.3K unique nodes are referenced, so the host compacts
    each core's table slice to <= 32768 rows and remaps endpoints to int16
    local ids, enabling the fast TIE-accelerated `dma_gather` row gather.
  - Edge i lives at SBUF (partition i%128, column i//128). Per chunk of
    kk*128 edges: dma_gather u-rows and v-rows, DVE adds (8-column pieces so
    the reducers start early), then per column a fused square+reduce -> norms,
    split ~50/50 between ScalarE (activation Square with accum_out) and
    VectorE (scalar_tensor_tensor self-multiply with accum_out) to balance
    engine load under the DMA chain. ScalarE applies sigmoid(beta*(x-mu)),
    VectorE multiplies by the edge weight; one final DMA stores all scores.
  - Chunk sizes descend toward the end so the compute drain after the last
    gather stays short; wp bufs=4 keeps the gather chain gapless.
"""

import os

import numpy as np

N_CORES = 8
N_NODES = 100000
D = 256
E_TOTAL = 300000
EPC = E_TOTAL // N_CORES          # 37500 edges per core
T = 293                           # 128-edge columns per core (37504 = 128*293)
EPAD = T * 128
NU_PAD = 32768                    # padded compacted-table rows (int16 id space)
CHUNKS = [int(x) for x in os.environ.get("ANOM_CHUNKS", "36,36,36,36,32,28,24,16,12,10,8,6,5,4,2,1,1").split(",")]
assert sum(CHUNKS) == T
K = max(CHUNKS)
BETA = 1.0
MU = 0.5
USE_BF16 = True
ACT_FRAC = float(os.environ.get("ANOM_ACT_FRAC", "0.50"))
N_QUEUES = 1                      # SWDGE queues: overlap desc-gen with transfers

_cache = {}


def _np_table_dtype():
    if USE_BF16:
        import ml_dtypes

        return ml_dtypes.bfloat16
    return np.float32


def _build_graph():
    import concourse.bacc as bacc
    import concourse.tile as tile
    from concourse import mybir

    f32 = mybir.dt.float32
    i16 = mybir.dt.int16
    dt = mybir.dt.bfloat16 if USE_BF16 else mybir.dt.float32

    nc = bacc.Bacc(num_swdge_queues=N_QUEUES)
    tab_u = nc.declare_dram_parameter("tab_u", [NU_PAD, D], dt, isOutput=False)
    tab_v = nc.declare_dram_parameter("tab_v", [NU_PAD, D], dt, isOutput=False)
    iu = nc.declare_dram_parameter("iu", [128, EPAD // 16], i16, isOutput=False)
    iv = nc.declare_dram_parameter("iv", [128, EPAD // 16], i16, isOutput=False)
    ws = nc.declare_dram_parameter("ws", [128, T], f32, isOutput=False)
    out = nc.declare_dram_parameter("out", [128, T], f32, isOutput=True)

    with tile.TileContext(nc) as tc:
        with (
            tc.tile_pool(name="io", bufs=1) as io,
            tc.tile_pool(name="wp", bufs=int(os.environ.get("ANOM_BUFS", "4"))) as wp,
        ):
            iu_t = io.tile([128, EPAD // 16], i16)
            iv_t = io.tile([128, EPAD // 16], i16)
            SPLIT = CHUNKS[0] * 8
            nc.sync.dma_start(out=iu_t[:, :SPLIT], in_=iu[:, :SPLIT])
            nc.sync.dma_start(out=iv_t[:, :SPLIT], in_=iv[:, :SPLIT])
            nc.sync.dma_start(out=iu_t[:, SPLIT:], in_=iu[:, SPLIT:])
            nc.sync.dma_start(out=iv_t[:, SPLIT:], in_=iv[:, SPLIT:])
            ws_t = io.tile([128, T], f32)
            nc.sync.dma_start(out=ws_t[:], in_=ws[:])
            out_t = io.tile([128, T], f32)
            bias_t = io.tile([128, 1], f32)
            nc.gpsimd.memset(bias_t[:], -BETA * MU)

            c0 = 0
            for kk in CHUNKS:
                c1 = c0 + kk
                n = kk * 128
                tu = wp.tile([128, K, D], dt, tag="tu")
                tv = wp.tile([128, K, D], dt, tag="tv")
                nc.gpsimd.dma_gather(
                    tu[:, :kk, :], tab_u[:], iu_t[:, c0 * 8 : c0 * 8 + n // 16],
                    n, n, D, single_packet=False,
                )
                nc.gpsimd.dma_gather(
                    tv[:, :kk, :], tab_v[:], iv_t[:, c0 * 8 : c0 * 8 + n // 16],
                    n, n, D, single_packet=False,
                )
                for s0 in range(0, kk, 8):
                    s1 = min(s0 + 8, kk)
                    nc.vector.tensor_tensor(
                        out=tu[:, s0:s1, :], in0=tu[:, s0:s1, :], in1=tv[:, s0:s1, :],
                        op=mybir.AluOpType.add,
                    )
                norm = wp.tile([128, K], f32, tag="norm")
                sq = wp.tile([128, D], dt, tag="sq")
                sqv = wp.tile([128, D], dt, tag="sqv")
                n_act = int(round(kk * ACT_FRAC))
                for j in range(kk):
                    if j < n_act:
                        nc.scalar.activation(
                            out=sq[:], in_=tu[:, j, :],
                            func=mybir.ActivationFunctionType.Square,
                            accum_out=norm[:, j : j + 1],
                        )
                    else:
                        nc.vector.scalar_tensor_tensor(
                            out=sqv[:], in0=tu[:, j, :], scalar=0.0,
                            in1=tu[:, j, :],
                            op0=mybir.AluOpType.add, op1=mybir.AluOpType.mult,
                            accum_out=norm[:, j : j + 1],
                        )
                nc.scalar.activation(
                    out=out_t[:, c0:c1], in_=norm[:, :kk],
                    func=mybir.ActivationFunctionType.Sigmoid,
                    scale=BETA, bias=bias_t[:],
                )
                nc.vector.tensor_tensor(
                    out=out_t[:, c0:c1], in0=out_t[:, c0:c1], in1=ws_t[:, c0:c1],
                    op=mybir.AluOpType.mult,
                )
                c0 = c1
            assert c0 == T
            nc.sync.dma_start(out=out[:], in_=out_t[:])
    nc.finalize()
    return nc


def _wrap_idx(idx16):
    """int16 [EPAD] -> [128, EPAD//16]; element j at [j%16, j//16], tiled x8."""
    w = idx16.reshape(EPAD // 16, 16).T
    return np.ascontiguousarray(np.tile(w, (8, 1)))


def _prepare_inputs(h, us, vs, ws, a, b):
    tdt = _np_table_dtype()
    h = np.asarray(h, dtype=np.float32)
    a = np.asarray(a, dtype=np.float32)
    b = np.asarray(b, dtype=np.float32)
    us = np.asarray(us).astype(np.int64, copy=False)
    vs = np.asarray(vs).astype(np.int64, copy=False)
    w = np.asarray(ws, dtype=np.float32)

    ha = (h * a[None, :]).astype(tdt)
    hb = (h * b[None, :]).astype(tdt)

    in_maps = []
    for c in range(N_CORES):
        sl = slice(c * EPC, (c + 1) * EPC)
        u = np.concatenate([us[sl], np.zeros(EPAD - EPC, np.int64)])
        v = np.concatenate([vs[sl], np.zeros(EPAD - EPC, np.int64)])
        wc = np.concatenate([w[sl], np.zeros(EPAD - EPC, np.float32)])

        uu, iu = np.unique(u, return_inverse=True)
        vv, iv = np.unique(v, return_inverse=True)
        if len(uu) > NU_PAD or len(vv) > NU_PAD:
            raise RuntimeError(
                f"core {c}: unique nodes {len(uu)}/{len(vv)} exceed int16 "
                f"table space {NU_PAD}"
            )
        tab_u = np.zeros((NU_PAD, D), dtype=tdt)
        tab_u[: len(uu)] = ha[uu]
        tab_v = np.zeros((NU_PAD, D), dtype=tdt)
        tab_v[: len(vv)] = hb[vv]

        in_maps.append(
            {
                "tab_u": tab_u,
                "tab_v": tab_v,
                "iu": _wrap_idx(iu.astype(np.int16)),
                "iv": _wrap_idx(iv.astype(np.int16)),
                "ws": np.ascontiguousarray(wc.reshape(T, 128).T),
            }
        )
    return in_maps


def kernel(h, us, vs, ws, a, b):
    from concourse.bass_utils import run_bass_kernel_spmd

    if "nc" not in _cache:
        _cache["nc"] = _build_graph()
    nc = _cache["nc"]

    in_maps = _prepare_inputs(h, us, vs, ws, a, b)
    res = run_bass_kernel_spmd(nc, in_maps, core_ids=list(range(N_CORES)))
    _cache["last_results"] = res

    outs = [
        res.results[c]["out"].T.ravel()[:EPC].astype(np.float32)
        for c in range(N_CORES)
    ]
    return np.concatenate(outs)
